# revision 37
# baseline (speedup 1.0000x reference)
"""Trainium2 Bass kernel for nn_DirectionalConvLayer.

Model (from the reference): per sample, a forward then backward scan over h;
each step = 3x3 conv on a single row (only the middle kernel row W[:,:,1,:]
contributes), + bias b, InstanceNorm over the row, ELU, + residual row. The
conv bias b cancels exactly under InstanceNorm and is never sent the device.

Sharding: data-parallel over batch n=8 -> one sample per NeuronCore, no
collectives. Each core runs the sequential 2*(h-1)-step scan on its sample.

Step math, with z = conv(prev row), m/v = row stats of z, rs = rsqrt(v+eps),
u = (z-m)*rs (standardized, so exp never overflows):
    f_dst = elu(u) + resid = [min(exp(u),1) - 1] + [relu(u)] + resid
          = em - 1 + ru*rs + resid
      em = exp(rs * min(z-m, 0)),  ru = max(z-m, 0)

Key structure (v2): the conv is linear, so the next step's conv input is kept
as a 128-partition "combo" slot holding em on partitions 0:64 and
q = ru*rs + (resid - 1) on partitions 64:128; one K=128 matmul per tap then
computes conv(em) + conv(q) together. The combo sums to f-1 (forward; pads
set to -1) or g+1 (backward; pads +1), making the conv result differ from
the true z only by a per-channel constant, which InstanceNorm's mean
subtraction absorbs exactly. This lets the three matmuls of step i+1 start
as soon as em/q land, while the fp32 stack write (clean f resp. g, used for
the backward residual and the final output) happens off the critical path.

All ACT functions used (Ln, Exp, Copy) live in one activation-table set
(natural_log_exp_and_others); the default chooser would reload tables twice
per step (~2.6us/step), so _Bacc pins the choice.

Per-step engine budget: PE 3 matmuls; DVE bn_stats/aggr + rd/ru (from an
SBUF copy of z, 2x perf mode) + q + stack write; ACT z-copy, Ln, Exp(rs),
Exp(em) -> combo and a duplicate em2 at partitions 64:128 so the stack write
reads same-base inputs (HW requires equal base partitions for multi-tensor
SBUF inputs; output base is free).
"""

from contextlib import ExitStack

import numpy as np

import concourse.bacc as bacc
import concourse.bass as bass
import concourse.mybir as mybir
import concourse.tile as tile
from concourse.bass_utils import run_bass_kernel_spmd

F32 = mybir.dt.float32
F16 = mybir.dt.float16
AF = mybir.ActivationFunctionType
OP = mybir.AluOpType

# fp16 combo slots: conv inputs (em/q) and weights in fp16 -> fast weight
# loads + cheaper matmuls; values are bounded (em in [0,1], |q| < ~20) so
# fp16 range is safe and its 10-bit mantissa keeps the end-to-end error
# ~3e-4. The fp32 stack (residual chain + output) is fed by fp32 duplicates
# (em2/q32), so only each step's elu-term is quantized.
COMBO_BF16 = True

EPS = 1e-5
C = 64          # channels
WDIM = 256      # row width
H_FULL = 256    # rows
SLOTW = WDIM + 2  # padded row slot width
XAHEAD = 4      # resid DMA prefetch distance
OUTB = 4        # out rows per store DMA
NCOMBO = 3      # combo ring depth
SBUFS = 3       # stats pool depth
USE_ZC = True   # stage z in SBUF via ACT copy (False: DVE reads PSUM direct)
ZDUP = False    # write z twice (2nd PSUM bank) to unserialize bank readers
ZC_MID = False  # emit the z-copy between Ln and Exp on ACT
EBUFS = 3       # elementwise pool depth
ZBUFS = 2       # z PSUM pool depth


class _Bacc(bacc.Bacc):
    """Bacc whose activation-table chooser is forced to the single set
    containing Ln and Exp (natural_log_exp_and_others); the default
    first-match rule alternates natural_log / exp_and_others, reloading
    ACT tables twice per scan step."""

    def insert_act_table_loads(self):
        import bass_rust as _bass_rust
        from concourse.hw_specs import get_activation_tables

        has_activation = any(
            isinstance(i, mybir.InstActivation)
            for b in self.main_func.blocks
            for i in b.instructions
        )
        if not has_activation:
            return
        want = {AF.Ln, AF.Exp, AF.Copy}
        tables = [
            (name, funcs if name == "natural_log_exp_and_others"
             else funcs - want)
            for name, funcs in get_activation_tables(self.m.arch).items()
        ]
        _bass_rust.insert_act_table_loads(self, tables)


def _build(h=H_FULL, combo_bf16=COMBO_BF16):
    cdt = F16 if combo_bf16 else F32
    half_rows = h // 2

    def hb(row):  # partition base of the stack half that owns `row`
        return 0 if row < half_rows else 64

    def soff(row):  # column offset of `row`'s slot within its half
        return (row % half_rows) * SLOTW

    nc = _Bacc("TRN2", target_bir_lowering=False, debug=False, num_devices=8)
    # xm1 = x - 1 (host-side); the -1 belongs inside q = ru*rs + resid - 1
    xm1 = nc.dram_tensor("xm1", [C, h, WDIM], F32, kind="ExternalInput").ap()
    # wb[half, ci, k, co] = W[co, ci, 1, k]; duplicated across both halves ->
    # combo matmuls contract em (parts 0:64) and q (parts 64:128) in one shot
    wb = nc.dram_tensor("wb", [2, C, 3, C], F32, kind="ExternalInput").ap()
    out = nc.dram_tensor("out", [C, h, WDIM], F32, kind="ExternalOutput").ap()

    with tile.TileContext(nc) as tc, ExitStack() as ctx:
        singles = ctx.enter_context(tc.tile_pool(name="singles", bufs=1))
        spool = ctx.enter_context(tc.tile_pool(name="stats", bufs=SBUFS))
        epool = ctx.enter_context(tc.tile_pool(name="elems", bufs=EBUFS))
        xpool = ctx.enter_context(tc.tile_pool(name="xrows", bufs=XAHEAD + 2))
        zpool = ctx.enter_context(tc.tile_pool(name="zpsum", bufs=ZBUFS, space="PSUM"))

        stack = singles.tile([128, half_rows * SLOTW], F32)
        w_both = singles.tile([128, 3 * C], F32)
        w_stage = singles.tile([128, 3 * C], F32)
        eps_t = singles.tile([128, 1], F32)
        nc.vector.memset(eps_t, EPS)

        # stage weights through a DVE copy so matmul weight deps are DVE ticks
        nc.sync.dma_start(out=w_stage, in_=wb)
        nc.vector.tensor_copy(w_both, w_stage)
        if combo_bf16:
            # fp16 weights for the steady (combo) taps; seeds stay fp32
            w16 = singles.tile([128, 3 * C], F16)
            nc.vector.tensor_copy(w16, w_stage)
        else:
            w16 = w_both

        # combo ring: persistent tiles. The em+q SUM over the two halves must
        # equal (f-1) resp. (g+1) per column including pads, so the phase pad
        # value lives on the q half only; em-half pads stay 0.
        combos = [singles.tile([128, SLOTW], cdt, name=f"combo{j}")
                  for j in range(NCOMBO)]
        for cb in combos:
            nc.vector.memset(cb[0:64, 0:1], 0.0)
            nc.vector.memset(cb[0:64, SLOTW - 1:SLOTW], 0.0)

        def set_pads(val):
            for cb in combos:
                nc.vector.memset(cb[64:128, 0:1], val)
                nc.vector.memset(cb[64:128, SLOTW - 1:SLOTW], val)

        # forward: q = ru*rs + (x-1) already carries elu's -1, so the combo
        # sums to exactly f -> pads stay 0 and the stack write adds 0
        set_pads(0.0)

        # stack pad columns are zero (seed rows are conv'd directly)
        stack3 = stack.rearrange("p (s c) -> p s c", c=SLOTW)
        nc.vector.memset(stack3[:, :, 0:1], 0.0)
        nc.vector.memset(stack3[:, :, SLOTW - 1:SLOTW], 0.0)

        # f[0] = x[0] = xm1[0] + 1, staged then fixed up on DVE
        x0 = xpool.tile([128, WDIM], F32, tag="xr")
        nc.sync.dma_start(out=x0[0:64, :], in_=xm1[:, 0, :])
        nc.vector.tensor_scalar_add(stack[0:64, 1:WDIM + 1], x0[0:64, :], 1.0)

        resid_tiles = {}

        def fetch_xrow(row):
            xr = xpool.tile([128, WDIM], F32, tag="xr", name=f"xr{row}")
            nc.sync.dma_start(out=xr[0:64, :], in_=xm1[:, row, :])
            resid_tiles[row] = xr

        def fetch_resid_bwd(row):
            # backward resid is stack row `row` (clean f); rows in the upper
            # half live at partitions 64:128 but the q-stt needs base-0
            # inputs -> stage through an SBUF->SBUF DMA. Lower half reads
            # the stack directly.
            if hb(row) == 0:
                resid_tiles[row] = None  # direct
                return
            xr = xpool.tile([128, WDIM], F32, tag="xr", name=f"br{row}")
            nc.sync.dma_start(
                out=xr[0:64, :],
                in_=stack[64:128, soff(row) + 1:soff(row) + 1 + WDIM],
            )
            resid_tiles[row] = xr

        def resid_ap(row):
            t = resid_tiles.pop(row)
            if t is None:
                return stack[0:64, soff(row) + 1:soff(row) + 1 + WDIM]
            return t[0:64, :]

        def step(dst, src_combo, src_stack_row, stack_scalar):
            """One scan step. Conv input: either a combo ring slot (K=128,
            em+q) or a stack row (seed steps, K=64). Writes em/q into
            combo[dst % NCOMBO] and the clean row into stack slot dst."""
            hd = hb(dst)
            do = soff(dst)
            zt = zpool.tile([128, WDIM], F32, tag="z", name=f"z{dst}")
            z = zt[0:64, :]
            ztargets = [z]
            if ZDUP:
                zt2 = zpool.tile([128, WDIM], F32, tag="zb", name=f"zb{dst}")
                ztargets.append(zt2[0:64, :])
            for zdst in ztargets:
                if src_combo is not None:
                    for k in range(3):
                        nc.tensor.matmul(
                            zdst,
                            lhsT=w16[:, k * C:(k + 1) * C],
                            rhs=src_combo[:, k:k + WDIM],
                            start=(k == 0),
                            stop=(k == 2),
                        )
                else:
                    hs = hb(src_stack_row)
                    so = soff(src_stack_row)
                    for k in range(3):
                        nc.tensor.matmul(
                            zdst,
                            lhsT=w_both[hs:hs + 64, k * C:(k + 1) * C],
                            rhs=stack[hs:hs + 64, so + k:so + k + WDIM],
                            start=(k == 0),
                            stop=(k == 2),
                        )
            z2 = ztargets[-1]

            st6 = spool.tile([128, 6], F32, tag="st6", name=f"st{dst}")
            nc.vector.bn_stats(st6[0:64, :], z)
            mv = spool.tile([128, 2], F32, tag="mv", name=f"mv{dst}")
            nc.vector.bn_aggr(mv[0:64, :], st6[0:64, :])
            mean = mv[0:64, 0:1]
            var = mv[0:64, 1:2]
            lv = spool.tile([128, 1], F32, tag="lv", name=f"lv{dst}")
            nc.scalar.activation(lv[0:64, :], var, AF.Ln, bias=eps_t[0:64, :])
            rs = spool.tile([128, 1], F32, tag="rs", name=f"rs{dst}")
            if USE_ZC and ZC_MID:
                zsrc_t = epool.tile([128, WDIM], F32, tag="zc", name=f"zc{dst}")
                nc.scalar.activation(zsrc_t[0:64, :], z2, AF.Copy)
                zsrc = zsrc_t[0:64, :]
            nc.scalar.activation(rs[0:64, :], lv[0:64, :], AF.Exp, scale=-0.5)
            if USE_ZC and not ZC_MID:
                # z copy to SBUF: unlocks DVE 2x perf mode for rd/ru, but
                # serializes them behind the ACT queue
                zsrc_t = epool.tile([128, WDIM], F32, tag="zc", name=f"zc{dst}")
                nc.scalar.activation(zsrc_t[0:64, :], z, AF.Copy)
                zsrc = zsrc_t[0:64, :]
            if not USE_ZC:
                zsrc = z2
            rd = epool.tile([128, WDIM], F32, tag="rd", name=f"rd{dst}")
            nc.vector.tensor_scalar(
                rd[0:64, :], zsrc, mean, 0.0, OP.subtract, OP.min
            )
            ru = epool.tile([128, WDIM], F32, tag="ru", name=f"ru{dst}")
            nc.vector.tensor_scalar(
                ru[0:64, :], zsrc, mean, 0.0, OP.subtract, OP.max
            )
            cb = combos[dst % NCOMBO]
            # em = exp(rs*min(z-m,0)) = min(exp(u),1), into combo em half
            nc.scalar.activation(
                cb[0:64, 1:WDIM + 1], rd[0:64, :], AF.Exp, scale=rs[0:64, :]
            )
            # q = ru*rs + resid(-1) into combo q half
            resid = resid_ap(dst)
            nc.vector.scalar_tensor_tensor(
                cb[64:128, 1:WDIM + 1], ru[0:64, :], rs[0:64, :],
                resid, OP.mult, OP.add,
            )
            # em duplicate at partitions 64:128 so the stack write reads
            # same-base inputs
            em2 = epool.tile([128, WDIM], F32, tag="em2", name=f"em2{dst}")
            nc.scalar.activation(
                em2[64:128, :], rd[0:64, :], AF.Exp, scale=rs[0:64, :]
            )
            if combo_bf16:
                # fp32 duplicate of q for the stack write (combo q is f16)
                qsrc = epool.tile([128, WDIM], F32, tag="q32", name=f"q32{dst}")
                nc.vector.scalar_tensor_tensor(
                    qsrc[64:128, :], ru[0:64, :], rs[0:64, :],
                    resid, OP.mult, OP.add,
                )
                qsrc = qsrc[64:128, :]
            else:
                qsrc = cb[64:128, 1:WDIM + 1]
            # stack <- em + q + stack_scalar (clean f fwd / clean g bwd), on
            # GPSIMD (plain TT / scalar-imm forms only; stt is rejected on
            # Pool by walrus) - SBUF-only and off the critical path
            fdst = stack[hd:hd + 64, do + 1:do + 1 + WDIM]
            if stack_scalar == 0.0:
                nc.gpsimd.tensor_tensor(fdst, em2[64:128, :], qsrc, OP.add)
            else:
                tmp = epool.tile([128, WDIM], F32, tag="fs", name=f"fs{dst}")
                nc.gpsimd.tensor_tensor(
                    tmp[64:128, :], em2[64:128, :], qsrc, OP.add
                )
                nc.gpsimd.tensor_scalar_add(fdst, tmp[64:128, :], stack_scalar)
            return cb

        # ---- forward scan: f[i] = elu-step(f[i-1]) + x[i] ----
        for rr in range(1, min(1 + XAHEAD, h)):
            fetch_xrow(rr)
        prev_combo = None
        for i in range(1, h):
            if i + XAHEAD <= h - 1:
                fetch_xrow(i + XAHEAD)
            # forward: resid = xm1 -> combo sums to f -> stack scalar 0
            prev_combo = step(
                i,
                prev_combo if i > 1 else None,
                i - 1,
                0.0,
            )

        # ---- backward scan: g[p] = elu-step(g[p+1]) + f[p] ----
        set_pads(1.0)  # backward: combo = g + 1

        def store_rows(p0):
            hd = hb(p0)
            src = stack[hd:hd + 64, :].rearrange("p (s c) -> p s c", c=SLOTW)
            s0 = soff(p0) // SLOTW
            nc.sync.dma_start(
                out=out[:, p0:p0 + OUTB, :],
                in_=src[:, s0:s0 + OUTB, 1:WDIM + 1],
            )

        for rr in range(h - 2, max(h - 2 - XAHEAD, -1), -1):
            fetch_resid_bwd(rr)
        prev_combo = None
        for p in range(h - 2, -1, -1):
            if p - XAHEAD >= 0:
                fetch_resid_bwd(p - XAHEAD)
            # backward: resid = clean f[p] -> combo sums to g+1 -> scalar -1
            prev_combo = step(
                p,
                prev_combo if p < h - 2 else None,
                p + 1,
                -1.0,
            )
            if p % OUTB == 0:
                store_rows(p)
        # the top store batch includes row h-1 (g[h-1] = f[h-1], from forward)
    nc.compile()
    return nc


_NC_CACHE = {}


def _get_nc(h=H_FULL):
    if h not in _NC_CACHE:
        _NC_CACHE[h] = _build(h)
    return _NC_CACHE[h]


def _in_maps(x, W):
    n = x.shape[0]
    w1t = W[:, :, 1, :].transpose(1, 2, 0)  # [ci, k, co]
    wb = np.ascontiguousarray(
        np.broadcast_to(w1t, (2,) + w1t.shape).astype(np.float32)
    )
    return [
        {
            "xm1": np.ascontiguousarray((x[s] - 1.0).astype(np.float32)),
            "wb": wb,
        }
        for s in range(n)
    ]


def run(x, W, h=H_FULL, **kw):
    nc = _get_nc(h)
    res = run_bass_kernel_spmd(
        nc, _in_maps(x, W), core_ids=list(range(x.shape[0])), **kw
    )
    outs = np.stack([r["out"] for r in res.results], axis=0)
    return outs, res


def kernel(x, W, b):
    x = np.asarray(x)
    W = np.asarray(W)
    outs, _ = run(x, W, h=x.shape[2])
    return outs.astype(np.float32)
